# revision 8
# baseline (speedup 1.0000x reference)
"""Bass/Trainium2 kernel for KnowledgeConsistentAttention (first-call forward).

Reference math (per image):
    kern = normalize(fg.reshape(C, H*W).T + eps)          # [P, C], P = H*W
    scores = kern @ fg.reshape(C, H*W)                    # [P, YX]
    scores = sum_pool3x3(scores over (y, x))
    att = softmax(scores, axis=1)
    out = kern.T @ att                                    # [C, YX]

Key identities used:
  * The 3x3 zero-padded sum pool acts on the RHS spatial axes only, so
    pool(kern @ fg) == kern @ pool(fg): pool the (tiny) input once instead
    of the (huge) scores.
  * softmax then kern.T @ att == (kern.T @ exp(s)) / (ones @ exp(s)):
    append a ones-column to kern so one matmul produces both numerator and
    denominator; divide at the end.  Scores are in [-30, 30] for this
    distribution, so exp() cannot overflow fp32 and no max-subtraction is
    needed.

Sharding: data-parallel, 8 cores = 4 images x 2 y-halves.  Per core the
steady state is a 64-stage pipeline (4 yx-chunks x 16 p-tile-pairs):
  GEMM1 (fp16) scores = kern_t.T @ fg2, two p-tiles packed into row-group
               halves of the PE array (K=64 each) -> concurrent.
  exp          split across two engines so neither is the bottleneck:
               8/16 stages on ScalarE (exact exp) and 8/16 on VectorE
               using a Schraudolph-style exp: i16 = int16(s*128*log2e +
               (127*128 - C)), bit-viewed as bf16 (~ +-3% rel).  The
               exact stages are chosen to cover the diagonal band of the
               attention matrix (p spatially near yx), which carries most
               of the softmax mass, keeping the end-to-end error ~8e-3.
  GEMM2 (bf16) one matmul per p-tile, M=128 (64 kern cols + ones col +
               63 zero cols so the weight load takes the fast FWL path)
               accumulating 32 p-tiles in one PSUM bank.
The PE slot for stage k is [G1_{k+2}, G2j1_{k-1}, G2j0_k]: GEMM1 runs two
slots ahead and each stage's second GEMM2 matmul is deferred one slot so
the exp-engine latency is covered within 3 PSUM score buffers.  Inputs
are loaded as independent slice-tiles on three DMA queues so the first
matmuls only wait for their own slice.  Host does the cheap prep
(normalize, pool, layouts) and the final divide.
"""

import numpy as np

B, C, H, W = 4, 64, 64, 64
P = H * W            # 4096 dynamic kernels (one per pixel)
YXH = (H // 2) * W   # 2048 output columns per core (half image)
EPS = 1e-7

NP_TILES = P // 128  # 32 p-tiles
NPAIRS = NP_TILES // 2
CHUNK = 512          # yx columns per psum bank
NCHUNK = YXH // CHUNK
OUTR = 65            # 64 channels + 1 ones-row (softmax denominator)
KAW = 128            # ka tile width (padded for FWL)

# Schraudolph exp in bf16 bit-space: exp(s) ~= bf16_bits(int16(s*A + Bc))
SCH_A = float(np.float32(128.0 / np.log(2.0)))   # 184.665...
SCH_B = float(127 * 128 - 6.0)                   # C=6 centers the rel err
N_ACT = 8            # exact-exp stages per 16-stage chunk (rest on DVE)

_CACHE = {}
G1DT = "float16"    # GEMM1 operand dtype (kt, rhs)
G2DT = "bfloat16"   # GEMM2 operand dtype (ka, e)
TRACE = False
LAST_RESULTS = None


def _chunk_order(ci):
    """Stage (pi) execution order for chunk ci and per-stage engine flags.

    Returns list of (pi, use_act).  ACT (exact exp) covers the diagonal
    band stages for both y-halves (p-tiles spatially within pool reach of
    the chunk's yx window), padded to N_ACT with the nearest remaining
    stages; the two lists are then merged strictly alternately so ScalarE
    and VectorE run concurrently.
    """
    prot = set()
    for h in (0, 1):
        for pi in range(8 * h + 2 * ci - 1, 8 * h + 2 * ci + 3):
            if 0 <= pi < 16:
                prot.add(pi)
    rest = [pi for pi in range(16) if pi not in prot]
    rest.sort(key=lambda pi: min(abs(pi - q) for q in prot))
    act = sorted(prot)
    while len(act) < N_ACT:
        act.append(rest.pop(0))
    act = act[:N_ACT]
    dve = sorted(pi for pi in range(16) if pi not in act)
    order = []
    for a, v in zip(act, dve):
        order.append((a, True))
        order.append((v, False))
    return order


def _build_program():
    import concourse.bacc as bacc
    import concourse.mybir as mybir
    import concourse.tile as tile
    from contextlib import ExitStack

    f32 = mybir.dt.float32
    i16 = mybir.dt.int16
    g1dt = getattr(mybir.dt, G1DT)
    g2dt = getattr(mybir.dt, G2DT)

    nc = bacc.Bacc("TRN2", target_bir_lowering=False, debug=False, num_devices=8)
    # kt2: pair layout — rows 0:64 even p-tiles, rows 64:128 odd p-tiles
    kt_d = nc.dram_tensor("kt2", [128, NPAIRS * 128], g1dt, kind="ExternalInput").ap()
    # ka2: per p-tile 128 cols (64 kern + ones + 63 zero), lhsT [K=128, M=128]
    ka_d = nc.dram_tensor("ka2", [128, NP_TILES * KAW], g2dt, kind="ExternalInput").ap()
    # rhs2: pooled fg half, duplicated into both row-group halves
    rhs_d = nc.dram_tensor("rhs2", [128, YXH], g1dt, kind="ExternalInput").ap()
    out_d = nc.dram_tensor("out65", [OUTR, YXH], f32, kind="ExternalOutput").ap()

    with tile.TileContext(nc) as tc, ExitStack() as ctx:
        const = ctx.enter_context(tc.tile_pool(name="const", bufs=1))
        # Separate tiles per DMA slice: readers then only wait for their
        # own slice (tile deps are whole-tile), and the three queues
        # (sync + scalar HWDGE, gpsimd SWDGE) run concurrently.
        kt_q = []
        rhs_c = []
        ka_s = []
        for qi in range(4):
            t = const.tile([128, 4 * 128], g1dt, name=f"ktq{qi}")
            nc.sync.dma_start(t[:], kt_d[:, qi * 512:(qi + 1) * 512])
            kt_q.append(t)
        for ci in range(NCHUNK):
            t = const.tile([128, CHUNK], g1dt, name=f"rhsc{ci}")
            nc.scalar.dma_start(t[:], rhs_d[:, ci * CHUNK:(ci + 1) * CHUNK])
            rhs_c.append(t)
        for si in range(8):
            t = const.tile([128, 4 * KAW], g2dt, name=f"kas{si}")
            nc.gpsimd.dma_start(t[:], ka_d[:, si * 4 * KAW:(si + 1) * 4 * KAW])
            ka_s.append(t)

        def kt_ap(pi, rows):
            return kt_q[pi // 4][rows, (pi % 4) * 128:(pi % 4 + 1) * 128]

        def ka_ap(t):
            return ka_s[t // 4][:, (t % 4) * KAW:(t % 4 + 1) * KAW]

        spool = ctx.enter_context(tc.tile_pool(name="spool", bufs=3, space="PSUM"))
        opool = ctx.enter_context(tc.tile_pool(name="opool", bufs=2, space="PSUM"))
        epool = ctx.enter_context(tc.tile_pool(name="epool", bufs=4))
        obuf = ctx.enter_context(tc.tile_pool(name="obuf", bufs=2))

        # Load the exp table set during the preamble so the first real
        # activation doesn't pay the ~1.3us ACT_TABLE_LOAD.
        warm = const.tile([128, 1], f32)
        nc.gpsimd.memset(warm[:], 0.0)
        nc.scalar.activation(warm[:], warm[:], mybir.ActivationFunctionType.Exp)

        stages = []
        for ci in range(NCHUNK):
            for k, (pi, use_act) in enumerate(_chunk_order(ci)):
                stages.append((ci, pi, use_act, k == 0, k == 15))
        s_tiles = [None] * len(stages)

        def emit_gemm1(k):
            ci, pi, _, _, _ = stages[k]
            s = spool.tile([128, 2 * CHUNK], f32, tag="s")
            s_tiles[k] = s
            nc.tensor.matmul(s[:, 0:CHUNK], kt_ap(pi, slice(0, 64)),
                             rhs_c[ci][0:64, :],
                             start=True, stop=True, tile_position=(0, 0))
            nc.tensor.matmul(s[:, CHUNK:2 * CHUNK], kt_ap(pi, slice(64, 128)),
                             rhs_c[ci][64:128, :],
                             start=True, stop=True, tile_position=(64, 0))

        def emit_j1(st):
            osum_p, ci_p, last_p, pi_p, e_p = st
            nc.tensor.matmul(
                osum_p[:, :], ka_ap(2 * pi_p + 1), e_p[:, CHUNK:2 * CHUNK],
                start=False, stop=last_p,
            )

        def emit_copy(cp):
            osum_p, ci_p = cp
            ob = obuf.tile([OUTR, CHUNK], f32, tag="ob")
            nc.vector.tensor_copy(ob[:], osum_p[0:OUTR, :])
            nc.sync.dma_start(out_d[:, ci_p * CHUNK:(ci_p + 1) * CHUNK], ob[:])

        osum = None
        prev = None
        pending = []  # (emit_at_k, (osum, ci)) chunk-end copies, deferred
        emit_gemm1(0)
        emit_gemm1(1)
        for k, (ci, pi, use_act, first, last) in enumerate(stages):
            if k + 2 < len(stages):
                emit_gemm1(k + 2)
            if prev is not None:
                emit_j1(prev)
                if prev[2]:  # chunk end: copy+DMA two slots later (DVE HOL)
                    pending.append((k + 2, (prev[0], prev[1])))
            while pending and pending[0][0] <= k:
                emit_copy(pending.pop(0)[1])
            if first:
                osum = opool.tile([128, CHUNK], f32, tag="osum")
            s = s_tiles[k]
            e = epool.tile([128, 2 * CHUNK], g2dt, tag="e")
            if use_act:
                nc.scalar.activation(e[:], s[:], mybir.ActivationFunctionType.Exp)
            else:
                nc.vector.tensor_scalar(
                    e[:].bitcast(i16), s[:], SCH_A, SCH_B,
                    op0=mybir.AluOpType.mult, op1=mybir.AluOpType.add)
            # p-tile 2*pi (kt2 rows 0:64 -> e slot 0), 2*pi+1 (slot 1)
            nc.tensor.matmul(
                osum[:, :], ka_ap(2 * pi), e[:, 0:CHUNK],
                start=first, stop=False,
            )
            s_tiles[k] = None
            prev = (osum, ci, last, pi, e)
        emit_j1(prev)
        emit_copy((prev[0], prev[1]))
    nc.compile()
    return nc


def _get_program():
    if "nc" not in _CACHE:
        _CACHE["nc"] = _build_program()
    return _CACHE["nc"]


def _pool3x3(x):
    # 3x3 stride-1 zero-padded sum pool over the last two axes.
    p = np.pad(x, ((0, 0), (0, 0), (1, 1), (0, 0)))
    x = p[:, :, :-2] + p[:, :, 1:-1] + p[:, :, 2:]
    p = np.pad(x, ((0, 0), (0, 0), (0, 0), (1, 1)))
    return p[:, :, :, :-2] + p[:, :, :, 1:-1] + p[:, :, :, 2:]


def _prep_inputs(foreground):
    import ml_dtypes

    _np_dt = {"bfloat16": ml_dtypes.bfloat16, "float16": np.float16,
              "float32r": np.float32}
    g1np, g2np = _np_dt[G1DT], _np_dt[G2DT]

    fg = np.ascontiguousarray(np.asarray(foreground, dtype=np.float32))
    assert fg.shape == (B, C, H, W)

    # kern_t[c, p] = normalized (fg + eps), kern transposed
    kt_all = fg.reshape(B, C, P) + EPS
    kt_all = kt_all / np.sqrt(
        (kt_all.astype(np.float64) ** 2).sum(1, keepdims=True)).astype(np.float32)
    # kt2: [128, NPAIRS*128] — even p-tiles in rows 0:64, odd in rows 64:128
    kt_r = kt_all.reshape(B, C, NPAIRS, 2, 128)
    kt2 = np.concatenate([kt_r[:, :, :, 0, :].reshape(B, C, NPAIRS * 128),
                          kt_r[:, :, :, 1, :].reshape(B, C, NPAIRS * 128)],
                         axis=1).astype(g1np)
    # ka2: [128, NP_TILES*128] — per p-tile 64 kern cols + ones + zero pad
    kq = kt_all.transpose(0, 2, 1).reshape(B, NP_TILES, 128, C)
    pad = np.zeros((B, NP_TILES, 128, KAW - C), np.float32)
    pad[..., 0] = 1.0
    kq = np.concatenate([kq, pad], -1)
    ka2 = np.ascontiguousarray(kq.transpose(0, 2, 1, 3)).reshape(
        B, 128, NP_TILES * KAW).astype(g2np)

    fg2 = _pool3x3(fg)

    in_maps = []
    for core in range(8):
        b, yh = core // 2, core % 2
        half = fg2[b, :, yh * (H // 2):(yh + 1) * (H // 2), :].reshape(C, YXH)
        in_maps.append({
            "kt2": np.ascontiguousarray(kt2[b]),
            "ka2": np.ascontiguousarray(ka2[b]),
            "rhs2": np.concatenate([half, half], axis=0).astype(g1np),
        })
    return in_maps


def kernel(foreground, masks=None, **_unused):
    global LAST_RESULTS
    from concourse import bass_utils

    in_maps = _prep_inputs(foreground)
    nc = _get_program()
    res = bass_utils.run_bass_kernel_spmd(
        nc, in_maps, core_ids=list(range(8)), trace=TRACE)
    LAST_RESULTS = res

    out = np.empty((B, C, H, W), dtype=np.float32)
    for core in range(8):
        b, yh = core // 2, core % 2
        oa = res.results[core]["out65"]  # [65, YXH]
        img = oa[0:C] / oa[C]
        out[b, :, yh * (H // 2):(yh + 1) * (H // 2), :] = img.reshape(C, H // 2, W)
    return out


# revision 11
# speedup vs baseline: 1.0206x; 1.0206x over previous
"""Bass/Trainium2 kernel for KnowledgeConsistentAttention (first-call forward).

Reference math (per image):
    kern = normalize(fg.reshape(C, H*W).T + eps)          # [P, C], P = H*W
    scores = kern @ fg.reshape(C, H*W)                    # [P, YX]
    scores = sum_pool3x3(scores over (y, x))
    att = softmax(scores, axis=1)
    out = kern.T @ att                                    # [C, YX]

Key identities used:
  * The 3x3 zero-padded sum pool acts on the RHS spatial axes only, so
    pool(kern @ fg) == kern @ pool(fg): pool the (tiny) input once instead
    of the (huge) scores.
  * softmax then kern.T @ att == (kern.T @ exp(s)) / (ones @ exp(s)):
    append a ones-column to kern so one matmul produces both numerator and
    denominator; divide at the end.  Scores are in [-30, 30] for this
    distribution, so exp() cannot overflow fp32 and no max-subtraction is
    needed.

Sharding: data-parallel, 8 cores = 4 images x 2 y-halves.  Per core the
steady state is a 64-stage pipeline (4 yx-chunks x 16 p-tile-pairs):
  GEMM1 (fp16) scores = kern_t.T @ fg2, two p-tiles packed into row-group
               halves of the PE array (K=64 each) -> concurrent.
  exp          split across two engines so neither is the bottleneck:
               8/16 stages on ScalarE (exact exp) and 8/16 on VectorE
               using a Schraudolph-style exp: i16 = int16(s*128*log2e +
               (127*128 - C)), bit-viewed as bf16 (~ +-3% rel).  The
               exact stages are chosen to cover the diagonal band of the
               attention matrix (p spatially near yx), which carries most
               of the softmax mass, keeping the end-to-end error ~8e-3.
  GEMM2 (bf16) one matmul per p-tile, M=128 (64 kern cols + ones col +
               63 zero cols so the weight load takes the fast FWL path)
               accumulating 32 p-tiles in one PSUM bank.
The PE slot for stage k is [G1_{k+2}, G2j1_{k-1}, G2j0_k]: GEMM1 runs two
slots ahead and each stage's second GEMM2 matmul is deferred one slot so
the exp-engine latency is covered within 3 PSUM score buffers.  Inputs
are loaded as independent slice-tiles on three DMA queues so the first
matmuls only wait for their own slice.  Host does the cheap prep
(normalize, pool, layouts) and the final divide.
"""

import numpy as np

B, C, H, W = 4, 64, 64, 64
P = H * W            # 4096 dynamic kernels (one per pixel)
YXH = (H // 2) * W   # 2048 output columns per core (half image)
EPS = 1e-7

NP_TILES = P // 128  # 32 p-tiles
NPAIRS = NP_TILES // 2
CHUNK = 512          # yx columns per psum bank
NCHUNK = YXH // CHUNK
OUTR = 65            # 64 channels + 1 ones-row (softmax denominator)
KAW = 128            # ka tile width (padded for FWL)

# Schraudolph exp in bf16 bit-space: exp(s) ~= bf16_bits(int16(s*A + Bc))
SCH_A = float(np.float32(128.0 / np.log(2.0)))   # 184.665...
SCH_B = float(127 * 128 - 6.0)                   # C=6 centers the rel err
N_ACT = 8            # exact-exp stages per 16-stage chunk (rest on DVE)

_CACHE = {}
G1DT = "float16"    # GEMM1 operand dtype (kt, rhs)
G2DT = "bfloat16"   # GEMM2 operand dtype (ka, e)
TRACE = False
LAST_RESULTS = None


def _chunk_order(ci):
    """Stage (pi) execution order for chunk ci and per-stage engine flags.

    Returns list of (pi, use_act).  ACT (exact exp) covers the diagonal
    band stages for both y-halves (p-tiles spatially within pool reach of
    the chunk's yx window), padded to N_ACT with the nearest remaining
    stages; the two lists are then merged strictly alternately so ScalarE
    and VectorE run concurrently.
    """
    prot = set()
    for h in (0, 1):
        for pi in range(8 * h + 2 * ci - 1, 8 * h + 2 * ci + 3):
            if 0 <= pi < 16:
                prot.add(pi)
    rest = [pi for pi in range(16) if pi not in prot]
    rest.sort(key=lambda pi: min(abs(pi - q) for q in prot))
    act = sorted(prot)
    while len(act) < N_ACT:
        act.append(rest.pop(0))
    act = act[:N_ACT]
    dve = sorted(pi for pi in range(16) if pi not in act)
    order = []
    for a, v in zip(act, dve):
        order.append((a, True))
        order.append((v, False))
    return order


def _build_program():
    import concourse.bacc as bacc
    import concourse.mybir as mybir
    import concourse.tile as tile
    from contextlib import ExitStack

    f32 = mybir.dt.float32
    i16 = mybir.dt.int16
    g1dt = getattr(mybir.dt, G1DT)
    g2dt = getattr(mybir.dt, G2DT)

    nc = bacc.Bacc("TRN2", target_bir_lowering=False, debug=False, num_devices=8)
    # kt2: pair layout — rows 0:64 even p-tiles, rows 64:128 odd p-tiles
    kt_d = nc.dram_tensor("kt2", [128, NPAIRS * 128], g1dt, kind="ExternalInput").ap()
    # ka2: per p-tile 128 cols (64 kern + ones + 63 zero), lhsT [K=128, M=128]
    ka_d = nc.dram_tensor("ka2", [128, NP_TILES * KAW], g2dt, kind="ExternalInput").ap()
    # rhs2: pooled fg half, duplicated into both row-group halves
    rhs_d = nc.dram_tensor("rhs2", [128, YXH], g1dt, kind="ExternalInput").ap()
    out_d = nc.dram_tensor("out65", [OUTR, YXH], f32, kind="ExternalOutput").ap()

    with tile.TileContext(nc) as tc, ExitStack() as ctx:
        const = ctx.enter_context(tc.tile_pool(name="const", bufs=1))
        # Separate tiles per DMA slice: readers then only wait for their
        # own slice (tile deps are whole-tile).  kt and ka ride the fast
        # gpsimd SWDGE queue (~130 GB/s, large descriptors), ordered by
        # first-use slot; rhs rides the scalar HWDGE queue.  A tiny
        # memset goes first on gpsimd so the exp-table-load warmup
        # activation has its input early.
        warm = const.tile([128, 1], f32)
        nc.gpsimd.memset(warm[:], 0.0)

        kt_q = [const.tile([128, 4 * 128], g1dt, name=f"ktq{qi}")
                for qi in range(4)]
        rhs_c = [const.tile([128, CHUNK], g1dt, name=f"rhsc{ci}")
                 for ci in range(NCHUNK)]
        ka_s = [const.tile([128, 4 * KAW], g2dt, name=f"kas{si}")
                for si in range(8)]

        def dma_kt(qi):
            nc.gpsimd.dma_start(kt_q[qi][:], kt_d[:, qi * 512:(qi + 1) * 512])

        def dma_ka(si):
            nc.gpsimd.dma_start(ka_s[si][:],
                                ka_d[:, si * 4 * KAW:(si + 1) * 4 * KAW])

        nc.scalar.dma_start(rhs_c[0][:], rhs_d[:, 0:CHUNK])
        nc.scalar.activation(warm[:], warm[:], mybir.ActivationFunctionType.Exp)
        for ci in range(1, NCHUNK):
            nc.scalar.dma_start(rhs_c[ci][:],
                                rhs_d[:, ci * CHUNK:(ci + 1) * CHUNK])
        dma_kt(0)
        dma_ka(0)
        dma_kt(1)
        dma_ka(2)
        dma_ka(1)
        dma_kt(2)
        dma_ka(3)
        dma_ka(5)
        dma_kt(3)
        dma_ka(4)
        dma_ka(6)
        dma_ka(7)

        def kt_ap(pi, rows):
            return kt_q[pi // 4][rows, (pi % 4) * 128:(pi % 4 + 1) * 128]

        def ka_ap(t):
            return ka_s[t // 4][:, (t % 4) * KAW:(t % 4 + 1) * KAW]

        spool = ctx.enter_context(tc.tile_pool(name="spool", bufs=3, space="PSUM"))
        opool = ctx.enter_context(tc.tile_pool(name="opool", bufs=2, space="PSUM"))
        epool = ctx.enter_context(tc.tile_pool(name="epool", bufs=4))
        obuf = ctx.enter_context(tc.tile_pool(name="obuf", bufs=2))

        stages = []
        for ci in range(NCHUNK):
            for k, (pi, use_act) in enumerate(_chunk_order(ci)):
                stages.append((ci, pi, use_act, k == 0, k == 15))
        s_tiles = [None] * len(stages)

        def emit_gemm1(k):
            ci, pi, _, _, _ = stages[k]
            s = spool.tile([128, 2 * CHUNK], f32, tag="s")
            s_tiles[k] = s
            nc.tensor.matmul(s[:, 0:CHUNK], kt_ap(pi, slice(0, 64)),
                             rhs_c[ci][0:64, :],
                             start=True, stop=True, tile_position=(0, 0))
            nc.tensor.matmul(s[:, CHUNK:2 * CHUNK], kt_ap(pi, slice(64, 128)),
                             rhs_c[ci][64:128, :],
                             start=True, stop=True, tile_position=(64, 0))

        def emit_j1(st):
            osum_p, ci_p, last_p, pi_p, e_p = st
            nc.tensor.matmul(
                osum_p[:, :], ka_ap(2 * pi_p + 1), e_p[:, CHUNK:2 * CHUNK],
                start=False, stop=last_p,
            )

        def emit_copy(cp):
            osum_p, ci_p = cp
            ob = obuf.tile([OUTR, CHUNK], f32, tag="ob")
            nc.vector.tensor_copy(ob[:], osum_p[0:OUTR, :])
            nc.gpsimd.dma_start(out_d[:, ci_p * CHUNK:(ci_p + 1) * CHUNK], ob[:])

        osum = None
        prev = None
        pending = []  # (emit_at_k, (osum, ci)) chunk-end copies, deferred
        emit_gemm1(0)
        emit_gemm1(1)
        emit_gemm1(2)
        for k, (ci, pi, use_act, first, last) in enumerate(stages):
            if k + 3 < len(stages):
                emit_gemm1(k + 3)
            if prev is not None:
                emit_j1(prev)
                if prev[2]:  # chunk end: copy+DMA two slots later (DVE HOL)
                    pending.append((k + 2, (prev[0], prev[1])))
            while pending and pending[0][0] <= k:
                emit_copy(pending.pop(0)[1])
            if first:
                osum = opool.tile([128, CHUNK], f32, tag="osum")
            s = s_tiles[k]
            e = epool.tile([128, 2 * CHUNK], g2dt, tag="e")
            if use_act:
                nc.scalar.activation(e[:], s[:], mybir.ActivationFunctionType.Exp)
            else:
                nc.vector.tensor_scalar(
                    e[:].bitcast(i16), s[:], SCH_A, SCH_B,
                    op0=mybir.AluOpType.mult, op1=mybir.AluOpType.add)
            # p-tile 2*pi (kt2 rows 0:64 -> e slot 0), 2*pi+1 (slot 1)
            nc.tensor.matmul(
                osum[:, :], ka_ap(2 * pi), e[:, 0:CHUNK],
                start=first, stop=False,
            )
            s_tiles[k] = None
            prev = (osum, ci, last, pi, e)
        emit_j1(prev)
        emit_copy((prev[0], prev[1]))
    nc.compile()
    return nc


def _get_program():
    if "nc" not in _CACHE:
        _CACHE["nc"] = _build_program()
    return _CACHE["nc"]


def _pool3x3(x):
    # 3x3 stride-1 zero-padded sum pool over the last two axes.
    p = np.pad(x, ((0, 0), (0, 0), (1, 1), (0, 0)))
    x = p[:, :, :-2] + p[:, :, 1:-1] + p[:, :, 2:]
    p = np.pad(x, ((0, 0), (0, 0), (0, 0), (1, 1)))
    return p[:, :, :, :-2] + p[:, :, :, 1:-1] + p[:, :, :, 2:]


def _prep_inputs(foreground):
    import ml_dtypes

    _np_dt = {"bfloat16": ml_dtypes.bfloat16, "float16": np.float16,
              "float32r": np.float32}
    g1np, g2np = _np_dt[G1DT], _np_dt[G2DT]

    fg = np.ascontiguousarray(np.asarray(foreground, dtype=np.float32))
    assert fg.shape == (B, C, H, W)

    # kern_t[c, p] = normalized (fg + eps), kern transposed
    kt_all = fg.reshape(B, C, P) + EPS
    kt_all = kt_all / np.sqrt(
        (kt_all.astype(np.float64) ** 2).sum(1, keepdims=True)).astype(np.float32)
    # kt2: [128, NPAIRS*128] — even p-tiles in rows 0:64, odd in rows 64:128
    kt_r = kt_all.reshape(B, C, NPAIRS, 2, 128)
    kt2 = np.concatenate([kt_r[:, :, :, 0, :].reshape(B, C, NPAIRS * 128),
                          kt_r[:, :, :, 1, :].reshape(B, C, NPAIRS * 128)],
                         axis=1).astype(g1np)
    # ka2: [128, NP_TILES*128] — per p-tile 64 kern cols + ones + zero pad
    kq = kt_all.transpose(0, 2, 1).reshape(B, NP_TILES, 128, C)
    pad = np.zeros((B, NP_TILES, 128, KAW - C), np.float32)
    pad[..., 0] = 1.0
    kq = np.concatenate([kq, pad], -1)
    ka2 = np.ascontiguousarray(kq.transpose(0, 2, 1, 3)).reshape(
        B, 128, NP_TILES * KAW).astype(g2np)

    fg2 = _pool3x3(fg)

    in_maps = []
    for core in range(8):
        b, yh = core // 2, core % 2
        half = fg2[b, :, yh * (H // 2):(yh + 1) * (H // 2), :].reshape(C, YXH)
        in_maps.append({
            "kt2": np.ascontiguousarray(kt2[b]),
            "ka2": np.ascontiguousarray(ka2[b]),
            "rhs2": np.concatenate([half, half], axis=0).astype(g1np),
        })
    return in_maps


def kernel(foreground, masks=None, **_unused):
    global LAST_RESULTS
    from concourse import bass_utils

    in_maps = _prep_inputs(foreground)
    nc = _get_program()
    res = bass_utils.run_bass_kernel_spmd(
        nc, in_maps, core_ids=list(range(8)), trace=TRACE)
    LAST_RESULTS = res

    out = np.empty((B, C, H, W), dtype=np.float32)
    for core in range(8):
        b, yh = core // 2, core % 2
        oa = res.results[core]["out65"]  # [65, YXH]
        img = oa[0:C] / oa[C]
        out[b, :, yh * (H // 2):(yh + 1) * (H // 2), :] = img.reshape(C, H // 2, W)
    return out


# revision 16
# speedup vs baseline: 1.0383x; 1.0173x over previous
"""Bass/Trainium2 kernel for KnowledgeConsistentAttention (first-call forward).

Reference math (per image):
    kern = normalize(fg.reshape(C, H*W).T + eps)          # [P, C], P = H*W
    scores = kern @ fg.reshape(C, H*W)                    # [P, YX]
    scores = sum_pool3x3(scores over (y, x))
    att = softmax(scores, axis=1)
    out = kern.T @ att                                    # [C, YX]

Key identities used:
  * The 3x3 zero-padded sum pool acts on the RHS spatial axes only, so
    pool(kern @ fg) == kern @ pool(fg): pool the (tiny) input once instead
    of the (huge) scores.
  * softmax then kern.T @ att == (kern.T @ exp(s)) / (ones @ exp(s)):
    append a ones-column to kern so one matmul produces both numerator and
    denominator; divide at the end.  Scores are in [-30, 30] for this
    distribution, so exp() cannot overflow fp32 and no max-subtraction is
    needed.

Sharding: data-parallel, 8 cores = 4 images x 2 y-halves.  Per core the
steady state is a 64-stage pipeline (4 yx-chunks x 16 p-tile-pairs):
  GEMM1 (fp16) scores = kern_t.T @ fg2, two p-tiles packed into row-group
               halves of the PE array (K=64 each) -> concurrent.
  exp          split across two engines so neither is the bottleneck:
               8/16 stages on ScalarE (exact exp) and 8/16 on VectorE
               using a Schraudolph-style exp: i16 = int16(s*128*log2e +
               (127*128 - C)), bit-viewed as bf16 (~ +-3% rel).  The
               exact stages are chosen to cover the diagonal band of the
               attention matrix (p spatially near yx), which carries most
               of the softmax mass, keeping the end-to-end error ~8e-3.
  GEMM2 (bf16) one matmul per p-tile, M=128 (64 kern cols + ones col +
               63 zero cols so the weight load takes the fast FWL path)
               accumulating 32 p-tiles in one PSUM bank.
The PE slot for stage k is [G1_{k+2}, G2j1_{k-1}, G2j0_k]: GEMM1 runs two
slots ahead and each stage's second GEMM2 matmul is deferred one slot so
the exp-engine latency is covered within 3 PSUM score buffers.  Inputs
are loaded as independent slice-tiles on three DMA queues so the first
matmuls only wait for their own slice.  Host does the cheap prep
(normalize, pool, layouts) and the final divide.
"""

import numpy as np

B, C, H, W = 4, 64, 64, 64
P = H * W            # 4096 dynamic kernels (one per pixel)
YXH = (H // 2) * W   # 2048 output columns per core (half image)
EPS = 1e-7

NP_TILES = P // 128  # 32 p-tiles
NPAIRS = NP_TILES // 2
CHUNK = 512          # yx columns per psum bank
NCHUNK = YXH // CHUNK
OUTR = 65            # 64 channels + 1 ones-row (softmax denominator)
KAW = 128            # ka tile width (padded for FWL)

# Schraudolph exp in bf16 bit-space: exp(s) ~= bf16_bits(int16(s*A + Bc))
SCH_A = float(np.float32(128.0 / np.log(2.0)))   # 184.665...
SCH_B = float(127 * 128 - 6.0)                   # C=6 centers the rel err
N_ACT = 8            # exact-exp stages per 16-stage chunk (rest on DVE)

_CACHE = {}
G1DT = "float16"    # GEMM1 operand dtype (kt, rhs)
G2DT = "bfloat16"   # GEMM2 operand dtype (ka, e)
TRACE = False
LAST_RESULTS = None


def _chunk_order(ci):
    """Stage (pi) execution order for chunk ci and per-stage engine flags.

    Returns list of (pi, use_act).  ACT (exact exp) covers the diagonal
    band stages for both y-halves (p-tiles spatially within pool reach of
    the chunk's yx window), padded to N_ACT with the nearest remaining
    stages; the two lists are then merged strictly alternately so ScalarE
    and VectorE run concurrently.
    """
    prot = set()
    for h in (0, 1):
        for pi in range(8 * h + 2 * ci - 1, 8 * h + 2 * ci + 3):
            if 0 <= pi < 16:
                prot.add(pi)
    rest = [pi for pi in range(16) if pi not in prot]
    rest.sort(key=lambda pi: min(abs(pi - q) for q in prot))
    act = sorted(prot)
    while len(act) < N_ACT:
        act.append(rest.pop(0))
    act = act[:N_ACT]
    dve = sorted(pi for pi in range(16) if pi not in act)
    order = []
    for a, v in zip(act, dve):
        order.append((a, True))
        order.append((v, False))
    return order


def _build_program():
    import concourse.bacc as bacc
    import concourse.mybir as mybir
    import concourse.tile as tile
    from contextlib import ExitStack

    f32 = mybir.dt.float32
    i16 = mybir.dt.int16
    g1dt = getattr(mybir.dt, G1DT)
    g2dt = getattr(mybir.dt, G2DT)

    nc = bacc.Bacc("TRN2", target_bir_lowering=False, debug=False, num_devices=8)
    # kt2: pair layout — rows 0:64 even p-tiles, rows 64:128 odd p-tiles
    kt_d = nc.dram_tensor("kt2", [128, NPAIRS * 128], g1dt, kind="ExternalInput").ap()
    # ka2: per p-tile 128 cols (64 kern + ones + 63 zero), lhsT [K=128, M=128]
    ka_d = nc.dram_tensor("ka2", [128, NP_TILES * KAW], g2dt, kind="ExternalInput").ap()
    # rhs2: pooled fg half, duplicated into both row-group halves
    rhs_d = nc.dram_tensor("rhs2", [128, YXH], g1dt, kind="ExternalInput").ap()
    out_d = nc.dram_tensor("out65", [OUTR, YXH], f32, kind="ExternalOutput").ap()

    with tile.TileContext(nc) as tc, ExitStack() as ctx:
        const = ctx.enter_context(tc.tile_pool(name="const", bufs=1))
        # Separate tiles per DMA slice: readers then only wait for their
        # own slice (tile deps are whole-tile).  kt and ka ride the fast
        # gpsimd SWDGE queue (~130 GB/s, large descriptors), ordered by
        # first-use slot; rhs rides the scalar HWDGE queue.  A tiny
        # memset goes first on gpsimd so the exp-table-load warmup
        # activation has its input early.
        warm = const.tile([128, 1], f32)
        nc.gpsimd.memset(warm[:], 0.0)

        # kt slice 0 is split in two so the very first GEMM1 starts early.
        kt_0a = const.tile([128, 256], g1dt, name="kt0a")
        kt_0b = const.tile([128, 256], g1dt, name="kt0b")
        kt_q = [None] + [const.tile([128, 4 * 128], g1dt, name=f"ktq{qi}")
                         for qi in range(1, 4)]
        rhs_c = [const.tile([128, CHUNK], g1dt, name=f"rhsc{ci}")
                 for ci in range(NCHUNK)]
        ka_s = [const.tile([128, 4 * KAW], g2dt, name=f"kas{si}")
                for si in range(8)]

        def dma_ka(si):
            nc.gpsimd.dma_start(ka_s[si][:],
                                ka_d[:, si * 4 * KAW:(si + 1) * 4 * KAW])

        # scalar HWDGE: first rhs chunk + kt slice 1, then the exp-table
        # warmup (loads during the DMA wait), then the late rhs chunks.
        nc.scalar.dma_start(rhs_c[0][:], rhs_d[:, 0:CHUNK])
        nc.scalar.dma_start(kt_q[1][:], kt_d[:, 512:1024])
        nc.scalar.activation(warm[:], warm[:], mybir.ActivationFunctionType.Exp)
        for ci in range(1, NCHUNK):
            nc.scalar.dma_start(rhs_c[ci][:],
                                rhs_d[:, ci * CHUNK:(ci + 1) * CHUNK])
        # gpsimd SWDGE: kt + ka in first-use order.
        nc.gpsimd.dma_start(kt_0a[:], kt_d[:, 0:256])
        nc.gpsimd.dma_start(kt_0b[:], kt_d[:, 256:512])
        dma_ka(0)
        dma_ka(2)
        dma_ka(1)
        nc.gpsimd.dma_start(kt_q[2][:], kt_d[:, 1024:1536])
        dma_ka(3)
        dma_ka(5)
        nc.gpsimd.dma_start(kt_q[3][:], kt_d[:, 1536:2048])
        dma_ka(4)
        dma_ka(6)
        dma_ka(7)

        def kt_ap(pi, rows):
            if pi < 2:
                return kt_0a[rows, (pi % 2) * 128:(pi % 2 + 1) * 128]
            if pi < 4:
                return kt_0b[rows, (pi % 2) * 128:(pi % 2 + 1) * 128]
            return kt_q[pi // 4][rows, (pi % 4) * 128:(pi % 4 + 1) * 128]

        def ka_ap(t):
            return ka_s[t // 4][:, (t % 4) * KAW:(t % 4 + 1) * KAW]

        spool = ctx.enter_context(tc.tile_pool(name="spool", bufs=3, space="PSUM"))
        opool = ctx.enter_context(tc.tile_pool(name="opool", bufs=2, space="PSUM"))
        epool = ctx.enter_context(tc.tile_pool(name="epool", bufs=4))

        stages = []
        for ci in range(NCHUNK):
            for k, (pi, use_act) in enumerate(_chunk_order(ci)):
                stages.append((ci, pi, use_act, k == 0, k == 15))
        s_tiles = [None] * len(stages)

        def emit_gemm1(k):
            ci, pi, _, _, _ = stages[k]
            s = spool.tile([128, 2 * CHUNK], f32, tag="s")
            s_tiles[k] = s
            nc.tensor.matmul(s[:, 0:CHUNK], kt_ap(pi, slice(0, 64)),
                             rhs_c[ci][0:64, :],
                             start=True, stop=True, tile_position=(0, 0))
            nc.tensor.matmul(s[:, CHUNK:2 * CHUNK], kt_ap(pi, slice(64, 128)),
                             rhs_c[ci][64:128, :],
                             start=True, stop=True, tile_position=(64, 0))

        def emit_j1(st):
            osum_p, ci_p, last_p, pi_p, e_p = st
            nc.tensor.matmul(
                osum_p[:, :], ka_ap(2 * pi_p + 1), e_p[:, CHUNK:2 * CHUNK],
                start=False, stop=last_p,
            )

        def emit_copy(cp):
            osum_p, ci_p = cp
            ob = epool.tile([OUTR, CHUNK], f32, tag="ob")
            nc.vector.tensor_copy(ob[:], osum_p[0:OUTR, :])
            nc.gpsimd.dma_start(out_d[:, ci_p * CHUNK:(ci_p + 1) * CHUNK], ob[:])

        osum = None
        prev = None
        pending = []  # (emit_at_k, (osum, ci)) chunk-end copies, deferred
        emit_gemm1(0)
        emit_gemm1(1)
        emit_gemm1(2)
        for k, (ci, pi, use_act, first, last) in enumerate(stages):
            if prev is not None:
                emit_j1(prev)
                if prev[2]:  # chunk end: copy+DMA two slots later (DVE HOL)
                    pending.append((k + 2, (prev[0], prev[1])))
            if k + 3 < len(stages):
                emit_gemm1(k + 3)
            while pending and pending[0][0] <= k:
                emit_copy(pending.pop(0)[1])
            if first:
                osum = opool.tile([128, CHUNK], f32, tag="osum")
            s = s_tiles[k]
            e = epool.tile([128, 2 * CHUNK], g2dt, tag="e")
            if use_act:
                nc.scalar.activation(e[:], s[:], mybir.ActivationFunctionType.Exp)
            else:
                nc.vector.tensor_scalar(
                    e[:].bitcast(i16), s[:], SCH_A, SCH_B,
                    op0=mybir.AluOpType.mult, op1=mybir.AluOpType.add)
            # p-tile 2*pi (kt2 rows 0:64 -> e slot 0), 2*pi+1 (slot 1)
            nc.tensor.matmul(
                osum[:, :], ka_ap(2 * pi), e[:, 0:CHUNK],
                start=first, stop=False,
            )
            s_tiles[k] = None
            prev = (osum, ci, last, pi, e)
        emit_j1(prev)
        emit_copy((prev[0], prev[1]))
    nc.compile()
    return nc


def _get_program():
    if "nc" not in _CACHE:
        _CACHE["nc"] = _build_program()
    return _CACHE["nc"]


def _pool3x3(x):
    # 3x3 stride-1 zero-padded sum pool over the last two axes.
    p = np.pad(x, ((0, 0), (0, 0), (1, 1), (0, 0)))
    x = p[:, :, :-2] + p[:, :, 1:-1] + p[:, :, 2:]
    p = np.pad(x, ((0, 0), (0, 0), (0, 0), (1, 1)))
    return p[:, :, :, :-2] + p[:, :, :, 1:-1] + p[:, :, :, 2:]


def _prep_inputs(foreground):
    import ml_dtypes

    _np_dt = {"bfloat16": ml_dtypes.bfloat16, "float16": np.float16,
              "float32r": np.float32}
    g1np, g2np = _np_dt[G1DT], _np_dt[G2DT]

    fg = np.ascontiguousarray(np.asarray(foreground, dtype=np.float32))
    assert fg.shape == (B, C, H, W)

    # kern_t[c, p] = normalized (fg + eps), kern transposed
    kt_all = fg.reshape(B, C, P) + EPS
    kt_all = kt_all / np.sqrt(
        (kt_all.astype(np.float64) ** 2).sum(1, keepdims=True)).astype(np.float32)
    # kt2: [128, NPAIRS*128] — even p-tiles in rows 0:64, odd in rows 64:128
    kt_r = kt_all.reshape(B, C, NPAIRS, 2, 128)
    kt2 = np.concatenate([kt_r[:, :, :, 0, :].reshape(B, C, NPAIRS * 128),
                          kt_r[:, :, :, 1, :].reshape(B, C, NPAIRS * 128)],
                         axis=1).astype(g1np)
    # ka2: [128, NP_TILES*128] — per p-tile 64 kern cols + ones + zero pad
    kq = kt_all.transpose(0, 2, 1).reshape(B, NP_TILES, 128, C)
    pad = np.zeros((B, NP_TILES, 128, KAW - C), np.float32)
    pad[..., 0] = 1.0
    kq = np.concatenate([kq, pad], -1)
    ka2 = np.ascontiguousarray(kq.transpose(0, 2, 1, 3)).reshape(
        B, 128, NP_TILES * KAW).astype(g2np)

    fg2 = _pool3x3(fg)

    in_maps = []
    for core in range(8):
        b, yh = core // 2, core % 2
        half = fg2[b, :, yh * (H // 2):(yh + 1) * (H // 2), :].reshape(C, YXH)
        in_maps.append({
            "kt2": np.ascontiguousarray(kt2[b]),
            "ka2": np.ascontiguousarray(ka2[b]),
            "rhs2": np.concatenate([half, half], axis=0).astype(g1np),
        })
    return in_maps


def kernel(foreground, masks=None, **_unused):
    global LAST_RESULTS
    from concourse import bass_utils

    in_maps = _prep_inputs(foreground)
    nc = _get_program()
    res = bass_utils.run_bass_kernel_spmd(
        nc, in_maps, core_ids=list(range(8)), trace=TRACE)
    LAST_RESULTS = res

    out = np.empty((B, C, H, W), dtype=np.float32)
    for core in range(8):
        b, yh = core // 2, core % 2
        oa = res.results[core]["out65"]  # [65, YXH]
        img = oa[0:C] / oa[C]
        out[b, :, yh * (H // 2):(yh + 1) * (H // 2), :] = img.reshape(C, H // 2, W)
    return out
